# revision 1
# baseline (speedup 1.0000x reference)
"""Multi-head attention (B=1, L=2048, D=1024, H=16) on 8 TRN2 NeuronCores.

Sharding: tensor-parallel over heads. Core i computes heads 2i, 2i+1:
  - projections with column shards of w_q/w_k/w_v (128 cols each)
  - full attention for its 2 heads
  - partial output projection with the matching 128-row shard of w_o
Host sums the 8 partial outputs (row-split w_concat => partial-sum combine).

On-device layout (all matmuls bf16, fp32 PSUM accumulate):
  - host ships q^T/k^T/v^T so the contraction dim (D) is on SBUF partitions;
    inputs land as single 2MB DMAs (fan out across all 16 SDMA engines)
  - projections emit qh^T/kh^T [128 = 2 heads x 64, L] directly
  - scores transposed: S^T[k,q] via lhsT=kh^T slice, rhs=qh^T; heads A/B in
    PE row groups 0/64 (K=64 matmuls pack); exp on ScalarE, scale=1/8 folded
  - P~ @ V col-group packed: head A -> PSUM partitions 0-63 via
    tile_position (0,0), head B -> 64-127 via (0,64); both heads share one
    [128, 1024] accumulator, which is already the concat^T layout
  - softmax denominators: P~ tiles accumulated on VectorE (bf16), column-sum
    via a ones-vector matmul; reciprocal computed partition-parallel after an
    SBUF spread DMA; broadcast back over 64 partitions with K=1 matmuls
  - out_partial (bf16) = concat_local @ wo_shard + b_o (bias added as a K=1
    ones matmul into the same PSUM group); host sums partials in fp32
"""

import os
import numpy as np
import ml_dtypes

import concourse.bass as bass
import concourse.mybir as mybir
import concourse.tile as tile
from concourse import bacc
from concourse.bass import ts
from concourse.bass_utils import run_bass_kernel_spmd
from concourse.masks import make_identity

P = 128
L = 2048
D = 1024
DH = 64
NCORES = 8
BF16 = mybir.dt.bfloat16
F32 = mybir.dt.float32
AF = mybir.ActivationFunctionType
ALU = mybir.AluOpType

TRACE = False  # test.py flips this to get an NTFF profile / exec_time_ns
LAST_RESULT = {}

_CACHED_NC = None


def _build():
    nc = bacc.Bacc("TRN2", target_bir_lowering=False, debug=False, num_devices=NCORES)

    qT = nc.dram_tensor("qT", [P, D // P, L], BF16, kind="ExternalInput")
    kT = nc.dram_tensor("kT", [P, D // P, L], BF16, kind="ExternalInput")
    vT = nc.dram_tensor("vT", [P, D // P, L], BF16, kind="ExternalInput")
    wq = nc.dram_tensor("wq", [P, D // P, P], BF16, kind="ExternalInput")
    wk = nc.dram_tensor("wk", [P, D // P, P], BF16, kind="ExternalInput")
    wv = nc.dram_tensor("wv", [P, D // P, P], BF16, kind="ExternalInput")
    bq = nc.dram_tensor("bq", [P, 1], F32, kind="ExternalInput")
    bk = nc.dram_tensor("bk", [P, 1], F32, kind="ExternalInput")
    bv = nc.dram_tensor("bv", [P, 1], F32, kind="ExternalInput")
    wo = nc.dram_tensor("wo", [P, D], BF16, kind="ExternalInput")
    bo = nc.dram_tensor("bo", [1, D], BF16, kind="ExternalInput")
    bo_bc = nc.dram_tensor("bo_bc", [P, D], F32, kind="ExternalInput")
    out = nc.dram_tensor("out", [L, D], BF16, kind="ExternalOutput")

    KT = D // P  # 8 contraction tiles for the projections
    LT = L // P  # 16 seq tiles

    with tile.TileContext(nc) as tc:
        with (
            tc.tile_pool(name="const", bufs=1) as const_pool,
            tc.tile_pool(name="inputs", bufs=1) as in_pool,
            tc.tile_pool(name="proj", bufs=1) as proj_pool,
            tc.tile_pool(name="work", bufs=1) as work_pool,
        ):
            identity = const_pool.tile([P, P], BF16)
            make_identity(nc, identity[:])
            ones_c = const_pool.tile([P, P], BF16)
            nc.vector.memset(ones_c[:], 1.0)
            scr = const_pool.tile([1, 32], F32)
            nc.scalar.activation(scr[:], ones_c[0:1, 0:32], AF.Exp)

            # ---- stage inputs: small tensors first, then one big DMA per
            # activation tensor (q on sync, k on scalar, v on gpsimd) ----
            wq_sb = in_pool.tile([P, KT, P], BF16)
            wk_sb = in_pool.tile([P, KT, P], BF16)
            wv_sb = in_pool.tile([P, KT, P], BF16)
            nc.sync.dma_start(wq_sb[:], wq[:])
            nc.scalar.dma_start(wk_sb[:], wk[:])
            nc.gpsimd.dma_start(wv_sb[:], wv[:])
            bq_sb = in_pool.tile([P, 1], F32)
            bk_sb = in_pool.tile([P, 1], F32)
            bv_sb = in_pool.tile([P, 1], F32)
            nc.sync.dma_start(bq_sb[:], bq[:])
            nc.scalar.dma_start(bk_sb[:], bk[:])
            nc.gpsimd.dma_start(bv_sb[:], bv[:])
            wo_sb = in_pool.tile([P, D], BF16)
            nc.gpsimd.dma_start(wo_sb[:], wo[:])
            bo_sb = in_pool.tile([1, D], BF16)
            nc.gpsimd.dma_start(bo_sb[:], bo[:])
            bo_bc_sb = in_pool.tile([P, D], F32)
            nc.gpsimd.dma_start(bo_bc_sb[:], bo_bc[:])

            qT_sb = in_pool.tile([P, KT, L], BF16)
            kT_sb = in_pool.tile([P, KT, L], BF16)
            vT_sb = in_pool.tile([P, KT, L], BF16)
            for dst, src in ((qT_sb, qT), (kT_sb, kT), (vT_sb, vT)):
                nc.sync.dma_start(dst[:, 0:3, :], src[:, 0:3, :])
                nc.scalar.dma_start(dst[:, 3:6, :], src[:, 3:6, :])
                nc.gpsimd.dma_start(dst[:, 6:8, :], src[:, 6:8, :])

            # ---- projections: qh^T / kh^T / vh^T  [128 (2 heads * 64), L] ----
            qhT = proj_pool.tile([P, L], BF16)
            khT = proj_pool.tile([P, L], BF16)
            vhT = proj_pool.tile([P, L], BF16)
            with tc.tile_pool(name="pp1", bufs=2, space="PSUM") as pp1:
                for w_sb, b_sb, x_sb, dst in (
                    (wq_sb, bq_sb, qT_sb, qhT),
                    (wk_sb, bk_sb, kT_sb, khT),
                    (wv_sb, bv_sb, vT_sb, vhT),
                ):
                    for n in range(L // 512):
                        ps = pp1.tile([P, 512], F32, tag="projps")
                        for t in range(KT):
                            nc.tensor.matmul(
                                ps[:],
                                w_sb[:, t, :],
                                x_sb[:, t, ts(n, 512)],
                                start=(t == 0),
                                stop=(t == KT - 1),
                            )
                        nc.vector.tensor_scalar(
                            dst[:, ts(n, 512)], ps[:], b_sb[:], None, op0=ALU.add
                        )

                # vh natural layout [kseq, 128]: cols 0:64 head A, 64:128 head B
                vh_sb = proj_pool.tile([P, LT, P], BF16)
                for t2 in range(LT):
                    pst = pp1.tile([P, P], BF16, tag="projps")
                    nc.tensor.transpose(pst[:], vhT[:, ts(t2, P)], identity[:])
                    nc.vector.tensor_copy(vh_sb[:, t2, :], pst[:])

            # ---- attention: heads in PE row groups (S^T) / col groups (AV),
            # qseq processed in halves of 1024 ----
            lhsT_c = work_pool.tile([P, L], BF16)  # normalized concat^T
            u_all = work_pool.tile([P, L], F32)  # unnormalized concat^T
            dall = work_pool.tile([1, 2, L], F32)  # denominators (partition 0)
            dallr = work_pool.tile([1, 2, L], BF16)  # 1/denominators
            dsp = work_pool.tile([P, 32], F32)
            dspb = work_pool.tile([P, 32], BF16)
            accs = {}
            with (
                tc.tile_pool(name="att_ps", bufs=1, space="PSUM") as att_ps,
                tc.tile_pool(name="pt_pool", bufs=2) as pt_pool,
                tc.tile_pool(name="acc_pool", bufs=2) as acc_pool,
            ):
                for qh in (0, 1):
                    av = att_ps.tile([P, 1024], F32, tag="av", name=f"av_{qh}")
                    for kt in range(LT):
                        pts = {}
                        sts = {}
                        for h in (0, 1):
                            st = att_ps.tile(
                                [P, 1024], F32, tag="st", bufs=3,
                                name=f"st{h}_{qh}_{kt}",
                            )
                            sts[h] = st
                        for j in (0, 1):
                            for h in (0, 1):
                                nc.tensor.matmul(
                                    sts[h][:, ts(j, 512)],
                                    khT[ts(h, DH), ts(kt, P)],
                                    qhT[ts(h, DH), qh * 1024 + j * 512 : qh * 1024 + (j + 1) * 512],
                                )
                        for h in (0, 1):
                            pt = pt_pool.tile(
                                [P, 1024], BF16, tag=f"pt{h}", name=f"pt{h}_{qh}_{kt}"
                            )
                            nc.scalar.activation(pt[:], sts[h][:], AF.Exp, scale=0.125)
                            pts[h] = pt
                        for j in (0, 1):
                            for h in (0, 1):
                                nc.tensor.matmul(
                                    av[ts(h, DH), ts(j, 512)],
                                    vh_sb[:, kt, ts(h, DH)],
                                    pts[h][:, ts(j, 512)],
                                    start=(kt == 0),
                                    stop=(kt == LT - 1),
                                    tile_position=(0, DH * h),
                                )
                        for h in (0, 1):
                            a = accs.get((h, qh))
                            if a is None:
                                a = acc_pool.tile(
                                    [P, 1024], BF16, tag=f"acc{h}", name=f"acc{h}_{qh}"
                                )
                                accs[(h, qh)] = a
                                nc.vector.tensor_copy(a[:], pts[h][:])
                            else:
                                nc.vector.tensor_tensor(
                                    a[:], a[:], pts[h][:], op=ALU.add
                                )
                    nc.vector.tensor_copy(u_all[:, ts(qh, 1024)], av[:])

            # ---- denominators: column-sum, spread, invert, broadcast ----
            with tc.tile_pool(name="fin_ps", bufs=1, space="PSUM") as fin_ps:
                for h in (0, 1):
                    for qh in (0, 1):
                        dcs = fin_ps.tile([1, 1024], F32, tag="dcs", name=f"dcs{h}{qh}")
                        for j in (0, 1):
                            nc.tensor.matmul(
                                dcs[:, ts(j, 512)],
                                ones_c[:, 0:1],
                                accs[(h, qh)][:, ts(j, 512)],
                            )
                        nc.vector.tensor_copy(dall[0:1, h, ts(qh, 1024)], dcs[:])
                # spread 4096 denominators across partitions, invert, put back
                nc.sync.dma_start(
                    dsp[:], dall[0:1, :, :].rearrange("a h q -> a (h q)")
                )
                nc.vector.reciprocal(dsp[:], dsp[:])
                nc.vector.tensor_copy(dspb[:], dsp[:])
                nc.sync.dma_start(
                    dallr[0:1, :, :].rearrange("a h q -> a (h q)"), dspb[:]
                )
                # broadcast 1/d over 64 partitions per head; scale u -> lhsT_c
                bc = fin_ps.tile([P, L], F32, tag="bc")
                for h in (0, 1):
                    for j2 in range(L // 512):
                        nc.tensor.matmul(
                            bc[ts(h, DH), ts(j2, 512)],
                            ones_c[0:1, 0:DH],
                            dallr[0:1, h, ts(j2, 512)],
                            tile_position=(0, DH * h),
                        )
                nc.vector.tensor_tensor(lhsT_c[:], u_all[:], bc[:], op=ALU.mult)

            # ---- output projection: out_partial = concat_local @ wo + b_o ----
            with (
                tc.tile_pool(name="op_ps", bufs=4, space="PSUM") as op_ps,
                tc.tile_pool(name="out_pool", bufs=3) as out_pool,
            ):
                for m in range(LT):
                    osb = out_pool.tile([P, D], BF16, tag="osb")
                    for n in (0, 1):
                        ps = op_ps.tile([P, 512], F32, tag="ops")
                        if n == 0:
                            nc.tensor.matmul(
                                ps[:], lhsT_c[:, ts(m, P)], wo_sb[:, ts(n, 512)]
                            )
                            nc.vector.tensor_tensor(
                                osb[:, ts(n, 512)],
                                ps[:],
                                bo_bc_sb[:, ts(n, 512)],
                                op=ALU.add,
                            )
                        else:
                            nc.tensor.matmul(
                                ps[:],
                                lhsT_c[:, ts(m, P)],
                                wo_sb[:, ts(n, 512)],
                                start=True,
                                stop=False,
                            )
                            nc.tensor.matmul(
                                ps[:],
                                ones_c[0:1, :],
                                bo_sb[0:1, ts(n, 512)],
                                start=False,
                                stop=True,
                            )
                            nc.scalar.copy(osb[:, ts(n, 512)], ps[:])
                    (nc.sync if m % 2 == 0 else nc.gpsimd).dma_start(
                        out[ts(m, P), :], osb[:]
                    )

    nc.compile()
    return nc


def kernel(q, k, v, w_q, b_q, w_k, b_k, w_v, b_v, w_o, b_o):
    global _CACHED_NC, LAST_RESULT
    if _CACHED_NC is None:
        _CACHED_NC = _build()
    nc = _CACHED_NC

    bf16 = ml_dtypes.bfloat16

    def tile_T(x):  # [L, D] -> [128, D//128, L] contiguous
        xt = np.asarray(x, np.float32)[0].T  # [D, L]
        return np.ascontiguousarray(
            xt.reshape(D // P, P, L).transpose(1, 0, 2)
        ).astype(bf16)

    def tile_w(w):  # [D, 128] -> [128, D//128, 128] contiguous
        return np.ascontiguousarray(
            w.reshape(D // P, P, P).transpose(1, 0, 2)
        ).astype(bf16)

    q2 = tile_T(q)
    k2 = tile_T(k)
    v2 = tile_T(v)
    w_q = np.asarray(w_q, np.float32)
    w_k = np.asarray(w_k, np.float32)
    w_v = np.asarray(w_v, np.float32)
    w_o = np.asarray(w_o, np.float32)
    b_q = np.asarray(b_q, np.float32)
    b_k = np.asarray(b_k, np.float32)
    b_v = np.asarray(b_v, np.float32)
    b_o = np.asarray(b_o, np.float32)

    in_maps = []
    for i in range(NCORES):
        sl = slice(P * i, P * (i + 1))
        bo_i = (
            b_o.reshape(1, D).astype(bf16) if i == 0 else np.zeros((1, D), bf16)
        )
        bo_bc_i = (
            np.ascontiguousarray(np.broadcast_to(b_o, (P, D))).astype(np.float32)
            if i == 0
            else np.zeros((P, D), np.float32)
        )
        in_maps.append(
            {
                "qT": q2,
                "kT": k2,
                "vT": v2,
                "wq": tile_w(w_q[:, sl]),
                "wk": tile_w(w_k[:, sl]),
                "wv": tile_w(w_v[:, sl]),
                "bq": np.ascontiguousarray(b_q[sl]).reshape(P, 1),
                "bk": np.ascontiguousarray(b_k[sl]).reshape(P, 1),
                "bv": np.ascontiguousarray(b_v[sl]).reshape(P, 1),
                "wo": np.ascontiguousarray(w_o[sl, :]).astype(bf16),
                "bo": bo_i,
                "bo_bc": bo_bc_i,
            }
        )

    kwargs = {}
    if TRACE:
        tdir = "/tmp/bass_trace"
        os.makedirs(tdir, exist_ok=True)
        kwargs["tmpdir"] = tdir
    res = run_bass_kernel_spmd(nc, in_maps, list(range(NCORES)), trace=TRACE, **kwargs)
    LAST_RESULT = {
        "exec_time_ns": res.exec_time_ns,
        "trace_path": (res.instructions_and_trace or (None, None))[1],
    }
    acc = np.zeros((L, D), np.float64)
    for i in range(NCORES):
        acc += res.results[i]["out"].astype(np.float64)
    return acc.astype(np.float32).reshape(1, L, D)

